# revision 28
# baseline (speedup 1.0000x reference)
"""Trainium2 Bass kernel for nn_EnhancedEdgeAwareGNN (edge-aware GAT, 6 layers).

Sharding: destination-node blocks (128 nodes) are assigned to 8 cores balanced
by in-edge count; each core aggregates all in-edges of its blocks (no
all-reduce), and one bf16 all-gather of h per layer republishes node features.
SPMD-uniform program: every core runs identical code; all per-core variation
lives in input tables (index tables, dstloc, permuted edge_attr).

Math reductions (exact): edge features enter only via al_e = w_edge @ Me
([128,24] per-layer reduction of lin_edge_w x att_edge); al_s/al_d likewise;
aggregation runs on h (128-d) with the per-head linear applied after; softmax
uses exp without max-subtraction (logits are O(1)); self-loops are dedicated
chunks fed by an all-reduced mean logit row (full sum + host pad correction).

Host-I/O and program size are minimized (the axon link is ~45MB/s with a
fixed ~14ms per-shard fetch latency): all shared weights ship bf16 packed
into one [128,X] blob sharded 1/8 per core and all-gathered on device, the
inner loops are hardware For_i loops with staggered resets (symbolic APs;
matmul lhsT offsets stay static), the output returns int8 with a per-row
f32 scale bitcast into the last 4 bytes of the same tensor (one output
tensor keeps the 8-shard fetch count minimal), the BIR serialization is
memoized on the nc instance, and the jax persistent compilation cache is
enabled so repeats skip the BIR->NEFF recompile.
"""

import sys

import numpy as np
import ml_dtypes

sys.path.insert(0, "/opt/trn_rl_repo")

BF = ml_dtypes.bfloat16

N, E, ND, ED, H, OUT, L, VC = 10000, 160000, 8, 4, 128, 256, 6, 6
HEADS, C = 4, 128
NCORES = 8
BLK = 128
NBLK_CORE = 10          # windows (128-node blocks) per core, padded
SPC = NBLK_CORE * BLK   # padded nodes per core (1280)
NPAD = NCORES * SPC     # padded rows in the replicated h table (10240)
GG = 4                  # edge chunks per P1 group

# bias_pack layout: gat_bias[L*H] | ln_scale[L*H] | ln_bias[L*H] | node_b[H]
#                   | out_b[OUT] | corr_row[L*HEADS]
BP_GB, BP_GS, BP_BB = 0, L * H, 2 * L * H
BP_NB, BP_OB = 3 * L * H, 3 * L * H + H
BP_CR = 3 * L * H + H + OUT
BP_TOT = BP_CR + L * HEADS  # 2712

# wpack layout (bf16, partition dim 128): lw | att1 | att2 | out_w | iota
#                                         | ident | me | msd | att3(pad)
WP_LW = 0
WP_A1 = WP_LW + L * HEADS * C   # 3072
WP_A2 = WP_A1 + H               # 3200
WP_OW = WP_A2 + 64              # 3264
WP_IO = WP_OW + OUT             # 3520
WP_ID = WP_IO + BLK             # 3648
WP_ME = WP_ID + BLK             # 3776
WP_MS = WP_ME + L * HEADS       # 3800
WP_A3 = WP_MS + L * 8           # 3848
WP_TOT = WP_A3 + 1              # 3849


# ----------------------------------------------------------------- host prep
def _split_blocks(dst):
    nblk = (N + BLK - 1) // BLK  # 79
    cnt = np.bincount(dst // BLK, minlength=nblk).astype(np.int64)
    cum = np.concatenate([[0], np.cumsum(cnt)])
    bounds = [0]
    for c in range(1, NCORES):
        target = cum[-1] * c / NCORES
        b = int(np.searchsorted(cum, target))
        lo = bounds[-1] + 1
        lo = max(lo, nblk - (NCORES - c) * NBLK_CORE)   # leave room behind
        hi = min(bounds[-1] + NBLK_CORE, nblk - (NCORES - c))
        bounds.append(max(lo, min(b, hi)))
    bounds.append(nblk)
    assert all(0 < bounds[i + 1] - bounds[i] <= NBLK_CORE for i in range(NCORES))
    return bounds


def _pad_coord(n, bounds):
    n = np.asarray(n)
    g = n // BLK
    c = np.searchsorted(np.asarray(bounds[1:]), g, side="right")
    return c * SPC + (g - np.asarray(bounds)[c]) * BLK + (n % BLK)


def _wrap16(idx):
    x = len(idx) // 16
    return np.ascontiguousarray(idx.reshape(x, 16).T.astype(np.int16))


def _build_graph(edge_index):
    src = np.asarray(edge_index[0], dtype=np.int64)
    dst = np.asarray(edge_index[1], dtype=np.int64)
    bounds = _split_blocks(dst)

    order = np.argsort(dst, kind="stable")
    src_s, dst_s = src[order], dst[order]
    blk_of = dst_s // BLK
    blk_starts = np.searchsorted(blk_of, np.arange(80))
    blk_ends = np.searchsorted(blk_of, np.arange(80), side="right")

    nblk = (N + BLK - 1) // BLK
    treg = max((blk_ends[g] - blk_starts[g] + BLK - 1) // BLK for g in range(nblk))
    T = treg + 1
    nregs = NBLK_CORE * treg
    # pad rc count to an even number of GG-groups (For_i step=2 over groups)
    ngrp = ((nregs + GG - 1) // GG + 1) // 2 * 2
    nregs_pad = ngrp * GG

    pc_src = _pad_coord(src_s, bounds)

    cores = []
    for c in range(NCORES):
        src_idx = np.zeros(NBLK_CORE * T * BLK, dtype=np.int64)
        dst_idx = np.zeros(NBLK_CORE * T * BLK, dtype=np.int64)
        dstloc_rc = np.full((BLK, nregs_pad), -1.0, dtype=np.float32)
        dstloc_sl = np.full((BLK, NBLK_CORE), -1.0, dtype=np.float32)
        ea_perm = np.zeros((nregs_pad * BLK,), dtype=np.int64)
        ea_mask = np.zeros((nregs_pad * BLK,), dtype=bool)
        for j in range(NBLK_CORE):
            g = bounds[c] + j
            real = g < bounds[c + 1]
            wbase = c * SPC + j * BLK
            cnt = (blk_ends[g] - blk_starts[g]) if real else 0
            s0 = blk_starts[g] if real else 0
            for k in range(treg):
                ch = j * T + k
                e0 = k * BLK
                take = max(0, min(BLK, cnt - e0))
                pos = ch * BLK
                if take:
                    sl = slice(s0 + e0, s0 + e0 + take)
                    src_idx[pos:pos + take] = pc_src[sl]
                    dst_idx[pos:pos + take] = wbase + (dst_s[sl] - g * BLK)
                    dstloc_rc[:take, j * treg + k] = (dst_s[sl] - g * BLK).astype(np.float32)
                    gp = (j * treg + k) * BLK
                    ea_perm[gp:gp + take] = order[sl]
                    ea_mask[gp:gp + take] = True
            # self-loop chunk: real nodes gather themselves, pads gather row 0
            ch = j * T + treg
            pos = ch * BLK
            ids = np.arange(BLK)
            nreal = min(BLK, max(0, N - g * BLK)) if real else 0
            coords = np.where(ids < nreal, wbase + ids, 0)
            src_idx[pos:pos + BLK] = coords
            dst_idx[pos:pos + BLK] = coords
            dstloc_sl[:nreal, j] = ids[:nreal].astype(np.float32)
        cores.append(dict(src_tab=_wrap16(src_idx), dst_tab=_wrap16(dst_idx),
                          dstloc_rc=dstloc_rc.astype(BF), dstloc_sl=dstloc_sl.astype(BF),
                          ea_perm=ea_perm, ea_mask=ea_mask))
    meta = dict(bounds=bounds, T=int(T), treg=int(treg),
                ngrp=int(ngrp), nregs_pad=int(nregs_pad))
    return cores, meta


def _derive_weights(inp, meta, cores):
    f32 = np.float32
    gw = {}
    lw = np.asarray(inp["gat_lin_w"], f32).reshape(L, H, HEADS, C)
    lew = np.asarray(inp["gat_lin_edge_w"], f32).reshape(L, H, HEADS, C)
    Ms = np.einsum("lkhc,lhc->lkh", lw, np.asarray(inp["gat_att_src"], f32))
    Md = np.einsum("lkhc,lhc->lkh", lw, np.asarray(inp["gat_att_dst"], f32))
    Me = np.einsum("lkhc,lhc->lkh", lew, np.asarray(inp["gat_att_edge"], f32))
    msd_f = np.ascontiguousarray(
        np.concatenate([Ms, Md], axis=2).transpose(1, 0, 2)).reshape(H, L * 8)
    me_f = np.ascontiguousarray(Me.transpose(1, 0, 2).reshape(H, L * HEADS))
    lw_full = np.ascontiguousarray(
        (np.asarray(inp["gat_lin_w"], f32) * 0.25).transpose(1, 0, 2)
    ).reshape(H, L * HEADS * C)                                            # [128,3072]
    vnf = np.asarray(inp["vnf_context"], f32) @ np.asarray(inp["vnf_w"], f32) \
        + np.asarray(inp["vnf_b"], f32)
    att1 = np.asarray(inp["att1_w"], f32)
    b1p_f = (np.asarray(inp["att1_b"], f32) + (vnf @ att1[H:]).ravel())
    att2_f = np.asarray(inp["att2_w"], f32)
    b2_f = np.asarray(inp["att2_b"], f32)
    att3_f = np.asarray(inp["att3_w"], f32)
    meta["b3"] = float(np.asarray(inp["att3_b"], f32).ravel()[0])
    eab_f = np.asarray(inp["ea_proj_b"], f32)
    gw["ea_proj_w"] = np.asarray(inp["ea_proj_w"], f32).astype(BF)
    gw["node_w"] = np.asarray(inp["node_w"], f32)

    ii = np.arange(BLK, dtype=f32)
    iota_f = np.tile(ii[None], (BLK, 1))
    att3_pad = np.zeros((BLK, 1), f32)
    att3_pad[:64, 0] = att3_f.ravel()

    wpack = np.zeros((BLK, WP_TOT), f32)
    wpack[:, WP_LW:WP_LW + L * HEADS * C] = lw_full
    wpack[:, WP_A1:WP_A1 + H] = att1[:H]
    wpack[:, WP_A2:WP_A2 + 64] = att2_f
    wpack[:, WP_OW:WP_OW + OUT] = np.asarray(inp["out_w"], f32)
    wpack[:, WP_IO:WP_IO + BLK] = iota_f
    wpack[:, WP_ID:WP_ID + BLK] = np.eye(BLK, dtype=f32)
    wpack[:, WP_ME:WP_ME + L * HEADS] = me_f
    wpack[:, WP_MS:WP_MS + L * 8] = msd_f
    wpack[:, WP_A3:WP_A3 + 1] = att3_pad
    wpack = wpack.astype(BF)

    colpack = np.zeros((BLK, 3), f32)
    colpack[:, 0] = b1p_f
    colpack[:, 1] = eab_f
    colpack[:64, 2] = b2_f
    gw["colpack"] = colpack

    # host pad-correction row: every pad slot in eaT contributes the same
    # al_e row (gate(eab-MLP) * (eab @ Me)); subtract count*row after the
    # cross-core allreduce of the full (unmasked) al_e column sum.
    a1 = np.maximum(att1[:H].T @ eab_f + b1p_f, 0.0)
    a2 = np.maximum(att2_f.T @ a1 + b2_f, 0.0)
    gate = 1.0 / (1.0 + np.exp(-(att3_f.ravel() @ a2 + meta["b3"])))
    r_pad = gate * (me_f.T @ eab_f)
    total_pads = NCORES * meta["nregs_pad"] * BLK - E

    gw["bias_pack"] = np.concatenate([
        np.asarray(inp["gat_bias"], f32).ravel(),
        np.asarray(inp["ln_scale"], f32).ravel(),
        np.asarray(inp["ln_bias"], f32).ravel(),
        np.asarray(inp["node_b"], f32).ravel(),
        np.asarray(inp["out_b"], f32).ravel(),
        (total_pads * r_pad).ravel(),
    ]).reshape(1, BP_TOT)

    bounds = meta["bounds"]
    x = np.asarray(inp["x"], f32)
    xT = np.zeros((ND, NPAD), f32)
    xT[:, _pad_coord(np.arange(N), bounds)] = x.T
    # per-core own x (local padded coords) for the fp32 h kept on-chip
    for c, cd in enumerate(cores):
        cd["xT_own"] = np.ascontiguousarray(xT[:, c * SPC:(c + 1) * SPC])
        buf = np.zeros((ED, meta["nregs_pad"] * BLK), BF)
        m = cd["ea_mask"]
        buf[:, m] = np.asarray(inp["edge_attr"], f32)[cd["ea_perm"][m]].T.astype(BF)
        cd["eaT"] = np.ascontiguousarray(
            np.concatenate([gw["ea_proj_w"], buf], axis=1))
        cd["wpack_sl"] = np.ascontiguousarray(wpack[16 * c:16 * (c + 1)])
    return gw


# --------------------------------------------------------------- bass kernel
def _build_kernel(meta):
    import os as _os
    STAGE = int(_os.environ.get("K_STAGE", "3"))
    SKIP = _os.environ.get("K_SKIP", "none")
    NL = int(_os.environ.get("K_NLAYERS", str(L)))
    import concourse.bass as bass
    import concourse.bacc as bacc
    import concourse.tile as tile
    from concourse import mybir
    ds = bass.ds

    F32, BF16, I16 = mybir.dt.float32, mybir.dt.bfloat16, mybir.dt.int16
    FP8 = mybir.dt.float8e4
    AF = mybir.ActivationFunctionType
    ALU = mybir.AluOpType
    T, treg = meta["T"], meta["treg"]
    ngrp, nregs_pad = meta["ngrp"], meta["nregs_pad"]
    B3 = meta["b3"]
    RG = [list(range(NCORES))]

    nc = bacc.Bacc(num_devices=NCORES)
    nc.has_collectives = True

    def ein(name, shape, dt=F32):
        return nc.dram_tensor(name, shape, dt, kind="ExternalInput")

    XTAB = NBLK_CORE * T * 8
    tabs_d = ein("tabs", [16, 2 * XTAB], I16)
    dl_d = ein("dstloc", [BLK, nregs_pad + NBLK_CORE], BF16)
    eaT_d = ein("eaT", [ED, H + nregs_pad * BLK], BF16)  # [eaw | eaT]
    xn_d = ein("xn", [ND, SPC + H])
    wpsl_d = ein("wpack_sl", [16, WP_TOT], BF16)
    bp_d = ein("bias_pack", [1, BP_TOT])
    cpk_d = ein("colpack", [BLK, 3])

    out_d = nc.dram_tensor("out", [SPC, OUT + 4], mybir.dt.int8, kind="ExternalOutput")

    h_table = nc.dram_tensor("h_table", [NPAD, H], BF16, addr_space="Shared")
    own_slice = nc.dram_tensor("own_slice", [SPC, H], BF16)
    wp_stage_t = nc.dram_tensor("wp_stage", [16, WP_TOT], BF16)
    wp_full_t = nc.dram_tensor("wp_full", [BLK, WP_TOT], BF16, addr_space="Shared")
    alel_in = nc.dram_tensor("alel_in", [1, L * HEADS], F32)
    alel_out = nc.dram_tensor("alel_out", [1, L * HEADS], F32, addr_space="Shared")

    with tile.TileContext(nc) as tc:
        with (
            tc.tile_pool(name="consts", bufs=1) as cp,
            tc.tile_pool(name="persist", bufs=1) as pers,
            tc.tile_pool(name="gath", bufs=2) as gp,
            tc.tile_pool(name="work", bufs=3) as wp,
            tc.tile_pool(name="small", bufs=4) as sp,
            tc.tile_pool(name="ps2", bufs=2, space="PSUM") as ps2,
            tc.tile_pool(name="ps1", bufs=1, space="PSUM") as ps1,
        ):
            def cload(dram, dt=None, name=None):
                t = cp.tile(dram.shape, dt or dram.dtype, name=name or (dram.name + "_sb"))
                nc.sync.dma_start(t[:], dram[:])
                return t

            # shared weight pack: sharded input -> allgather -> [128, WP_TOT]
            # (collectives cannot read IO tensors; bounce via internal DRAM)
            nc.sync.dma_start(wp_stage_t[:], wpsl_d[:])
            nc.gpsimd.collective_compute(
                "AllGather", ALU.bypass, replica_groups=RG,
                ins=[wp_stage_t[:]], outs=[wp_full_t[:]])
            wpk = cp.tile([BLK, WP_TOT], BF16, name="wpk")
            nc.sync.dma_start(wpk[:], wp_full_t[:])
            att1 = wpk[:, WP_A1:WP_A1 + H]
            att2 = wpk[:, WP_A2:WP_A2 + 64]
            outw = wpk[:, WP_OW:WP_OW + OUT]
            me = wpk[:, WP_ME:WP_ME + L * HEADS]
            att3 = wpk[0:64, WP_A3:WP_A3 + 1]

            # gather index tables: ship [16,X], tile 8x across partitions here
            src_tab = cp.tile([BLK, XTAB], I16, name="src_tab_sb")
            dst_tab = cp.tile([BLK, XTAB], I16, name="dst_tab_sb")
            for k in range(8):
                nc.sync.dma_start(src_tab[16 * k:16 * (k + 1), :], tabs_d[:, 0:XTAB])
                nc.sync.dma_start(dst_tab[16 * k:16 * (k + 1), :], tabs_d[:, XTAB:2 * XTAB])

            dl = cload(dl_d)
            dlrc = dl[:, 0:nregs_pad]
            dlsl = dl[:, nregs_pad:nregs_pad + NBLK_CORE]
            xn = cload(xn_d)
            xTo = xn[:, 0:SPC]
            nw = xn[:, SPC:SPC + H]
            eaw = cp.tile([ED, H], BF16, name="eaw_sb")
            nc.sync.dma_start(eaw[:], eaT_d[:, 0:H])
            bpk = cload(bp_d)
            cpk = cload(cpk_d)
            b1p = cpk[:, 0:1]
            eab = cpk[:, 1:2]
            b2 = cpk[0:64, 2:3]

            iota = cp.tile([BLK, BLK], F32, name="iota_f")
            nc.vector.tensor_copy(iota[:], wpk[:, WP_IO:WP_IO + BLK])
            ident = cp.tile([BLK, BLK], F32, name="ident_f")
            nc.vector.tensor_copy(ident[:], wpk[:, WP_ID:WP_ID + BLK])

            for cv in (0.0, 1e-5, 1e-30, B3):
                ct = cp.tile([BLK, 1], F32, name=f"const_{abs(hash(cv)) % 10**8}")
                nc.vector.memset(ct[:], cv)
                nc.const_aps.aps[(F32, cv)] = ct[:]

            ones_bf = cp.tile([BLK, 1], BF16, name="ones_bf")
            nc.vector.memset(ones_bf[:], 1.0)
            ones_col = cp.tile([BLK, 1], F32, name="ones_col")
            nc.vector.memset(ones_col[:], 1.0)
            one_f = cp.tile([1, 1], F32, name="one_f")
            nc.vector.memset(one_f[:], 1.0)
            ones_row = cp.tile([1, BLK], F32, name="ones_row")
            nc.vector.memset(ones_row[:], 1.0)

            # broadcast bias_pack [1,2712] -> [128,2712] via K=1 matmuls
            brd = cp.tile([BLK, BP_TOT], F32, name="brd")
            for j in range(0, BP_TOT, 448):
                wdt = min(448, BP_TOT - j)
                bp_ps = ps2.tile([BLK, 448], F32, name="bp_ps", tag="big", bufs=3)
                nc.tensor.matmul(bp_ps[:, :wdt], ones_row[:], bpk[:, j:j + wdt],
                                 start=True, stop=True)
                nc.vector.tensor_copy(brd[:, j:j + wdt], bp_ps[:, :wdt])

            h_own = pers.tile([BLK, NBLK_CORE * H], F32, name="h_own")
            al_e = pers.tile([BLK, nregs_pad, L * HEADS], F32, name="al_e")
            alel_sb = pers.tile([BLK, L * HEADS], F32, name="alel_sb")
            acc24 = pers.tile([BLK, GG, L * HEADS], F32, name="acc24")
            nc.vector.memset(acc24[:], 0.0)

            # ---- P0: initial embedding of own blocks; AllGather -> h_table
            for w in range(NBLK_CORE):
                h0p = ps2.tile([BLK, 4, H], F32, name="hps", tag="big", bufs=3)
                nc.tensor.matmul(h0p[:, 0, :], xTo[:, w * BLK:(w + 1) * BLK], nw,
                                 start=True, stop=True)
                nc.vector.tensor_tensor(out=h_own[:, w * H:(w + 1) * H], in0=h0p[:, 0, :],
                                        in1=brd[:, BP_NB:BP_NB + H], op=ALU.add)
                h0b = wp.tile([BLK, H], BF16, name="h0b", tag="h0b")
                nc.vector.tensor_copy(h0b[:], h_own[:, w * H:(w + 1) * H])
                nc.sync.dma_start(own_slice[w * BLK:(w + 1) * BLK, :], h0b[:])
            nc.gpsimd.collective_compute(
                "AllGather", ALU.bypass, replica_groups=RG,
                ins=[own_slice[:]], outs=[h_table[:]])

            # ---- P1: edge gate MLP -> al_e table; full sum -> corr -> allreduce
            if STAGE < 1:
                nc.vector.memset(al_e[:], 0.0)
                nc.vector.memset(alel_sb[:], 0.0)
            if STAGE >= 1:
                def p1_body(jv):
                    rc0 = jv * GG
                    ea_t = wp.tile([ED, GG * BLK], BF16, name="ea_t", tag="ea_t")
                    nc.sync.dma_start(ea_t[:], eaT_d[:, ds(H + rc0 * BLK, GG * BLK)])
                    efp = ps2.tile([BLK, GG * BLK], F32, name="efp", tag="big", bufs=3)
                    nc.tensor.matmul(efp[:], eaw[:], ea_t[:], start=True, stop=True)
                    efb = wp.tile([BLK, GG * BLK], BF16, name="efb", tag="efb")
                    nc.scalar.activation(efb[:], efp[:], AF.Identity, bias=eab)
                    a1p = ps2.tile([BLK, GG * BLK], F32, name="a1p", tag="big", bufs=3)
                    nc.tensor.matmul(a1p[:], att1, efb[:], start=True, stop=True)
                    a1 = wp.tile([BLK, GG * BLK], BF16, name="a1", tag="a1")
                    nc.scalar.activation(a1[:], a1p[:], AF.Relu, bias=b1p)
                    a2p = ps2.tile([64, GG * BLK], F32, name="a2p", tag="big", bufs=3)
                    nc.tensor.matmul(a2p[:], att2, a1[:], start=True, stop=True)
                    a2 = wp.tile([64, GG * BLK], BF16, name="a2", tag="a2")
                    nc.scalar.activation(a2[:], a2p[:], AF.Relu, bias=b2)
                    for q in range(GG):
                        gcp = ps2.tile([BLK, 32], F32, name="gcp", tag="big", bufs=3)
                        nc.tensor.matmul(gcp[:, 0:1], a2[:, q * BLK:(q + 1) * BLK], att3,
                                         start=True, stop=True)
                        gcol = sp.tile([BLK, 1], F32, name="gcol", tag="gcol")
                        nc.scalar.activation(gcol[:], gcp[:, 0:1], AF.Sigmoid, bias=B3)
                        pfxp = ps2.tile([BLK, 32], F32, name="pfxp", tag="big", bufs=3)
                        nc.tensor.matmul(pfxp[:, 0:L * HEADS], efb[:, q * BLK:(q + 1) * BLK],
                                         me, start=True, stop=True)
                        nc.vector.tensor_scalar(out=al_e[:, ds(rc0 + q, 1), :],
                                                in0=pfxp[:, 0:L * HEADS],
                                                scalar1=gcol[:, 0:1], scalar2=None,
                                                op0=ALU.mult)
                    nc.vector.tensor_tensor(out=acc24[:], in0=acc24[:],
                                            in1=al_e[:, ds(rc0, GG), :], op=ALU.add)

                with tc.For_i(0, ngrp, 2) as jv:
                    p1_body(jv)
                    p1_body(jv + 1)

                # fold acc24 [128,GG,24] -> [1,24], subtract pad corr, allreduce
                accf = sp.tile([BLK, L * HEADS], F32, name="accf")
                nc.vector.tensor_tensor(out=accf[:], in0=acc24[:, 0, :],
                                        in1=acc24[:, 1, :], op=ALU.add)
                for qq in (2, 3):
                    nc.vector.tensor_tensor(out=accf[:], in0=accf[:],
                                            in1=acc24[:, qq, :], op=ALU.add)
                alel_ps = ps1.tile([1, L * HEADS], F32, name="alel_ps", tag="alel")
                nc.tensor.matmul(alel_ps[:], ones_col[:], accf[:], start=True, stop=True)
                alel_row = sp.tile([1, L * HEADS], F32, name="alel_row")
                nc.vector.tensor_copy(alel_row[:], alel_ps[:])
                nc.sync.dma_start(alel_in[:], alel_row[:])
                nc.gpsimd.collective_compute(
                    "AllReduce", ALU.add, replica_groups=RG,
                    ins=[alel_in[:]], outs=[alel_out[:]])
                alel_row2 = sp.tile([1, L * HEADS], F32, name="alel_row2")
                nc.sync.dma_start(alel_row2[:], alel_out[:])
                nc.vector.tensor_tensor(out=alel_row2[:], in0=alel_row2[:],
                                        in1=brd[0:1, BP_CR:BP_CR + L * HEADS],
                                        op=ALU.subtract)
                alel_bp = ps1.tile([BLK, L * HEADS], F32, name="alel_bp", tag="alel")
                nc.tensor.matmul(alel_bp[:], ones_row[:], alel_row2[:], start=True, stop=True)
                nc.vector.tensor_scalar(out=alel_sb[:], in0=alel_bp[:], scalar1=1.0 / E,
                                        scalar2=None, op0=ALU.mult)

            # ---- P2: GAT layers (hardware loop over windows, 2x unrolled)
            niw_reg = nc.gpsimd.alloc_register()
            nc.gpsimd.reg_mov(niw_reg, T * BLK)
            for li in range(NL if STAGE >= 2 else 0):
                l = li % L
                last = (li == NL - 1)

                def p2_body(wv, l=l, last=last):
                    dstw = sp.tile([BLK, T], F32, name="dstw", tag="dstw")
                    nc.vector.tensor_copy(dstw[:, 0:treg], dlrc[:, ds(wv * treg, treg)])
                    nc.vector.tensor_copy(dstw[:, treg:T], dlsl[:, ds(wv, 1)])
                    hg = gp.tile([BLK, T, H], BF16, name="hg", tag="hg")
                    hgt = gp.tile([BLK, 1, T * BLK], BF16, name="hgt", tag="hgt")
                    hdt = gp.tile([BLK, 1, T * BLK], BF16, name="hdt", tag="hdt")
                    ssl = src_tab[:, ds(wv * (T * 8), T * 8)]
                    dsl = dst_tab[:, ds(wv * (T * 8), T * 8)]
                    ni = T * BLK
                    if SKIP != "gath":
                        nc.gpsimd.dma_gather(out_ap=hg[:, :, :], in_ap=h_table[:, :],
                                             idxs_ap=ssl, num_idxs=ni, num_idxs_reg=niw_reg,
                                             elem_size=H, single_packet=False)
                        nc.gpsimd.dma_gather(out_ap=hgt[:, :, :], in_ap=h_table[:, :],
                                             idxs_ap=ssl, num_idxs=ni, num_idxs_reg=niw_reg,
                                             elem_size=H, transpose=True, single_packet=False)
                        nc.gpsimd.dma_gather(out_ap=hdt[:, :, :], in_ap=h_table[:, :],
                                             idxs_ap=dsl, num_idxs=ni, num_idxs_reg=niw_reg,
                                             elem_size=H, transpose=True, single_packet=False)
                    if SKIP == "body":
                        return
                    alpha_ps = ps2.tile([BLK, 4 * T], F32, name="alpha_ps", tag="big", bufs=3)
                    for k in range(T):
                        nc.tensor.matmul(alpha_ps[:, k * 4:(k + 1) * 4],
                                         hgt[:, 0, k * BLK:(k + 1) * BLK],
                                         wpk[:, WP_MS + l * 8:WP_MS + l * 8 + 4],
                                         start=True, stop=False)
                        nc.tensor.matmul(alpha_ps[:, k * 4:(k + 1) * 4],
                                         hdt[:, 0, k * BLK:(k + 1) * BLK],
                                         wpk[:, WP_MS + l * 8 + 4:WP_MS + l * 8 + 8],
                                         start=False, stop=True)
                    t_sb = wp.tile([BLK, 4 * T], F32, name="t_sb", tag="t_sb")
                    nc.vector.tensor_tensor(
                        out=t_sb[:, 0:4 * treg].rearrange("p (t f) -> p t f", f=4),
                        in0=alpha_ps[:, 0:4 * treg].rearrange("p (t f) -> p t f", f=4),
                        in1=al_e[:, ds(wv * treg, treg), l * 4:(l + 1) * 4],
                        op=ALU.add)
                    nc.vector.tensor_tensor(out=t_sb[:, 4 * treg:4 * T],
                                            in0=alpha_ps[:, 4 * treg:4 * T],
                                            in1=alel_sb[:, l * 4:(l + 1) * 4], op=ALU.add)
                    u_sb = wp.tile([BLK, 4 * T], F32, name="u_sb", tag="u_sb")
                    nc.scalar.activation(u_sb[:], t_sb[:], AF.Lrelu, alpha=0.2)
                    ex_sb = wp.tile([BLK, 4 * T], F32, name="ex_sb", tag="ex_sb")
                    nc.scalar.activation(ex_sb[:], u_sb[:], AF.Exp)

                    numT_ps = ps2.tile([BLK, HEADS * BLK], F32, name="numT_ps",
                                       tag="numT", bufs=2)
                    den_ps = ps2.tile([1, HEADS * BLK], F32, name="den_ps", tag="den", bufs=1)
                    for k in range(T):
                        eq = wp.tile([BLK, BLK], F32, name="eq", tag="eq")
                        nc.vector.tensor_tensor(out=eq[:],
                                                in0=dstw[:, k:k + 1].to_broadcast([BLK, BLK]),
                                                in1=iota[:], op=ALU.is_equal)
                        sw = wp.tile([BLK, HEADS, BLK], BF16, name="sw", tag="sw")
                        for hd in range(HEADS):
                            nc.vector.tensor_tensor(
                                out=sw[:, hd, :], in0=eq[:],
                                in1=ex_sb[:, k * 4 + hd:k * 4 + hd + 1].to_broadcast([BLK, BLK]),
                                op=ALU.mult)
                        nc.tensor.matmul(numT_ps[:], hg[:, k, :], sw[:, :, :],
                                         start=(k == 0), stop=(k == T - 1))
                        nc.tensor.matmul(den_ps[:], ones_bf[:], sw[:, :, :],
                                         start=(k == 0), stop=(k == T - 1))

                    numT_sb = wp.tile([BLK, HEADS * BLK], BF16, name="numT_sb", tag="numsb")
                    nc.vector.tensor_copy(numT_sb[:], numT_ps[:])
                    den_sb = sp.tile([1, HEADS * BLK], F32, name="den_sb", tag="densb")
                    nc.vector.tensor_copy(den_sb[:], den_ps[:])
                    denT_ps = ps2.tile([BLK, 4], F32, name="denT_ps", tag="big", bufs=3)
                    for hd in range(HEADS):
                        nc.tensor.matmul(denT_ps[:, hd:hd + 1],
                                         den_sb[:, hd * BLK:(hd + 1) * BLK], one_f[:],
                                         start=True, stop=True)
                    dr = sp.tile([BLK, 4], F32, name="dr", tag="dr")
                    nc.vector.tensor_scalar(out=dr[:], in0=denT_ps[:], scalar1=1e-30,
                                            scalar2=None, op0=ALU.add)
                    nc.vector.reciprocal(dr[:], dr[:])

                    hc_ps = ps2.tile([BLK, HEADS, BLK], F32, name="hc_ps", tag="big", bufs=3)
                    for hd in range(HEADS):
                        nc.tensor.matmul(hc_ps[:, hd, :],
                                         numT_sb[:, hd * BLK:(hd + 1) * BLK],
                                         wpk[:, WP_LW + l * 512 + hd * C:
                                             WP_LW + l * 512 + (hd + 1) * C],
                                         start=True, stop=True)
                    acc = wp.tile([BLK, BLK], F32, name="acc", tag="acc")
                    nc.vector.tensor_scalar(out=acc[:], in0=hc_ps[:, 0, :],
                                            scalar1=dr[:, 0:1], scalar2=None, op0=ALU.mult)
                    for hd in range(1, HEADS):
                        tmp = sp.tile([BLK, BLK], F32, name="tmp", tag="tmp")
                        nc.vector.tensor_scalar(out=tmp[:], in0=hc_ps[:, hd, :],
                                                scalar1=dr[:, hd:hd + 1], scalar2=None,
                                                op0=ALU.mult)
                        nc.vector.tensor_tensor(out=acc[:], in0=acc[:], in1=tmp[:], op=ALU.add)
                    nc.vector.tensor_tensor(out=acc[:], in0=acc[:],
                                            in1=brd[:, BP_GB + l * H:BP_GB + (l + 1) * H],
                                            op=ALU.add)
                    nc.scalar.activation(acc[:], acc[:], AF.Relu)
                    r = wp.tile([BLK, BLK], F32, name="r", tag="r")
                    nc.vector.tensor_tensor(out=r[:], in0=acc[:],
                                            in1=h_own[:, ds(wv * H, H)], op=ALU.add)
                    # LayerNorm over features
                    s1 = sp.tile([BLK, 1], F32, name="s1", tag="s1")
                    nc.vector.tensor_reduce(s1[:], r[:], axis=mybir.AxisListType.X, op=ALU.add)
                    negm = sp.tile([BLK, 1], F32, name="negm", tag="negm")
                    nc.scalar.activation(negm[:], s1[:], AF.Copy, scale=-1.0 / H)
                    xc = wp.tile([BLK, BLK], F32, name="xc", tag="xc")
                    nc.vector.tensor_scalar(out=xc[:], in0=r[:], scalar1=negm[:, 0:1],
                                            scalar2=None, op0=ALU.add)
                    sq = wp.tile([BLK, BLK], F32, name="sq", tag="sq")
                    vs = sp.tile([BLK, 1], F32, name="vs", tag="vs")
                    nc.scalar.activation(sq[:], xc[:], AF.Square, accum_out=vs[:])
                    std = sp.tile([BLK, 1], F32, name="std", tag="std")
                    nc.scalar.activation(std[:], vs[:], AF.Sqrt, scale=1.0 / H, bias=1e-5)
                    rstd = sp.tile([BLK, 1], F32, name="rstd", tag="rstd")
                    nc.vector.reciprocal(rstd[:], std[:])
                    nc.vector.tensor_scalar(out=xc[:], in0=xc[:], scalar1=rstd[:, 0:1],
                                            scalar2=None, op0=ALU.mult)
                    nc.vector.tensor_tensor(out=xc[:], in0=xc[:],
                                            in1=brd[:, BP_GS + l * H:BP_GS + (l + 1) * H],
                                            op=ALU.mult)
                    nc.vector.tensor_tensor(out=h_own[:, ds(wv * H, H)], in0=xc[:],
                                            in1=brd[:, BP_BB + l * H:BP_BB + (l + 1) * H],
                                            op=ALU.add)
                    if not last:
                        hb = wp.tile([BLK, H], BF16, name="hb", tag="hb")
                        nc.vector.tensor_copy(hb[:], h_own[:, ds(wv * H, H)])
                        nc.sync.dma_start(own_slice[ds(wv * BLK, BLK), :], hb[:])

                with tc.For_i(0, NBLK_CORE, 2, staggered_reset=True) as wv:
                    p2_body(wv)
                    p2_body(wv + 1)

                if SKIP != "coll" and not last:
                    nc.gpsimd.collective_compute(
                        "AllGather", ALU.bypass, replica_groups=RG,
                        ins=[own_slice[:]], outs=[h_table[:]])

            # ---- P3: output projection (node-major rows per core)
            for w in range(NBLK_CORE):
                tp = ps2.tile([BLK, BLK], F32, name="tp", tag="big", bufs=3)
                nc.tensor.transpose(tp[:], h_own[:, w * H:(w + 1) * H], ident[:])
                hT = wp.tile([BLK, BLK], BF16, name="hT", tag="hT")
                nc.vector.tensor_copy(hT[:], tp[:])
                op_ = ps2.tile([BLK, OUT], F32, name="op_", tag="numT", bufs=2)
                nc.tensor.matmul(op_[:], hT[:], outw, start=True, stop=True)
                o_sb = wp.tile([BLK, OUT], F32, name="o_sb", tag="o_sb")
                nc.vector.tensor_tensor(out=o_sb[:], in0=op_[:],
                                        in1=brd[:, BP_OB:BP_OB + OUT], op=ALU.add)
                # int8 quantize, per-row scale embedded in the last 4 bytes
                osq = wp.tile([BLK, OUT], F32, name="osq", tag="osq")
                nc.scalar.activation(osq[:], o_sb[:], AF.Square)
                om2 = sp.tile([BLK, 1], F32, name="om2", tag="om2")
                nc.vector.tensor_reduce(om2[:], osq[:], axis=mybir.AxisListType.X,
                                        op=ALU.max)
                omp = sp.tile([BLK, 1], F32, name="omp", tag="omp")
                nc.scalar.activation(omp[:], om2[:], AF.Sqrt, bias=1e-30)
                orm = sp.tile([BLK, 1], F32, name="orm", tag="orm")
                nc.vector.reciprocal(orm[:], omp[:])
                qf = wp.tile([BLK, OUT], F32, name="qf", tag="qf")
                nc.vector.tensor_scalar(out=qf[:], in0=o_sb[:], scalar1=orm[:, 0:1],
                                        scalar2=127.0, op0=ALU.mult, op1=ALU.mult)
                q8 = wp.tile([BLK, OUT], mybir.dt.int8, name="q8", tag="q8")
                nc.vector.tensor_copy(q8[:], qf[:])
                nc.sync.dma_start(out_d[w * BLK:(w + 1) * BLK, 0:OUT], q8[:])
                osc = sp.tile([BLK, 1], F32, name="osc", tag="osc")
                nc.vector.tensor_scalar(out=osc[:], in0=omp[:], scalar1=1.0 / 127,
                                        scalar2=None, op0=ALU.mult)
                nc.sync.dma_start(out_d[w * BLK:(w + 1) * BLK, OUT:OUT + 4],
                                  osc[:].bitcast(mybir.dt.int8))

    nc.compile()
    return nc


# -------------------------------------------------------------------- driver
_KCACHE = {}
_LAST_IN_MAPS = None


def _setup_jax_cache():
    try:
        import jax
        jax.config.update("jax_compilation_cache_dir", "/tmp/jax_pcc")
        jax.config.update("jax_persistent_cache_min_compile_time_secs", 0.0)
        jax.config.update("jax_persistent_cache_min_entry_size_bytes", 0)
    except Exception:
        pass


def kernel(x, edge_index, edge_attr, vnf_context, node_w, node_b, ea_proj_w, ea_proj_b,
           vnf_w, vnf_b, att1_w, att1_b, att2_w, att2_b, att3_w, att3_b,
           gat_lin_w, gat_att_src, gat_att_dst, gat_lin_edge_w, gat_att_edge, gat_bias,
           ln_scale, ln_bias, out_w, out_b):
    _setup_jax_cache()
    from concourse.bass_utils import run_bass_kernel_spmd

    inp = dict(x=x, edge_index=edge_index, edge_attr=edge_attr, vnf_context=vnf_context,
               node_w=node_w, node_b=node_b, ea_proj_w=ea_proj_w, ea_proj_b=ea_proj_b,
               vnf_w=vnf_w, vnf_b=vnf_b, att1_w=att1_w, att1_b=att1_b, att2_w=att2_w,
               att2_b=att2_b, att3_w=att3_w, att3_b=att3_b, gat_lin_w=gat_lin_w,
               gat_att_src=gat_att_src, gat_att_dst=gat_att_dst,
               gat_lin_edge_w=gat_lin_edge_w, gat_att_edge=gat_att_edge,
               gat_bias=gat_bias, ln_scale=ln_scale, ln_bias=ln_bias,
               out_w=out_w, out_b=out_b)

    cores, meta = _build_graph(edge_index)
    gw = _derive_weights(inp, meta, cores)

    shared = {k: np.ascontiguousarray(v) for k, v in gw.items() if k != "ea_proj_w"}
    nw_shared = shared.pop("node_w")
    shared["node_w"] = nw_shared  # keep for xn concat below
    in_maps = []
    for c in range(NCORES):
        m = {k: v for k, v in shared.items() if k != "node_w"}
        m["tabs"] = np.ascontiguousarray(
            np.concatenate([cores[c]["src_tab"], cores[c]["dst_tab"]], axis=1))
        m["dstloc"] = np.ascontiguousarray(
            np.concatenate([cores[c]["dstloc_rc"], cores[c]["dstloc_sl"]], axis=1))
        m["eaT"] = cores[c]["eaT"]
        m["xn"] = np.ascontiguousarray(
            np.concatenate([cores[c]["xT_own"], shared["node_w"]], axis=1))
        m["wpack_sl"] = cores[c]["wpack_sl"]
        in_maps.append(m)

    key = (meta["T"], meta["nregs_pad"], meta["b3"])
    if key not in _KCACHE:
        nc_new = _build_kernel(meta)
        # the module is immutable after compile(); memoize the per-call
        # BIR serialization (~35ms) that bass_exec lowering re-runs each call
        _jb = nc_new.to_json_bytes()
        nc_new.to_json_bytes = lambda _b=_jb: _b
        _KCACHE[key] = nc_new
    nc = _KCACHE[key]

    global _LAST_IN_MAPS
    _LAST_IN_MAPS = in_maps
    res = run_bass_kernel_spmd(nc, in_maps, list(range(NCORES)))
    bounds = meta["bounds"]
    out = np.zeros((N, OUT), dtype=np.float32)
    for c in range(NCORES):
        r0, r1 = bounds[c] * BLK, min(bounds[c + 1] * BLK, N)
        raw = np.asarray(res.results[c]["out"][:r1 - r0])
        q = raw[:, :OUT].astype(np.float32)
        s = np.ascontiguousarray(raw[:, OUT:OUT + 4]).view(np.float32)
        out[r0:r1] = q * s
    return out
